# revision 1
# baseline (speedup 1.0000x reference)
"""Pairwise cosine-similarity kernel for Trainium2 (8 NeuronCores, SPMD).

Computes out = 16 * normalize(x1) @ normalize(x2).T for x1, x2 [8192, 512] f32.

Sharding: x1 rows are split across the 8 cores (1024 rows each); x2 is
replicated. Each core computes its [1024, 8192] slice of the output; the host
concatenates the slices and upcasts bf16 -> f32.

Host-side prep is layout/dtype only:
  - x1t [512, 1024] bf16: pre-transposed x1 slice (GEMM stationary source).
  - x2t [512, 8192] bf16: pre-transposed x2 (GEMM moving source).
  - x1n [2, 128, 2048] fp8e4m3, x2n [16, 128, 2048] fp8e4m3: natural-layout
    row-grouped copies used only for row-norm computation
    (group g holds rows g*512 + j*128 + p at [g, p, j*512:(j+1)*512]).

All FLOPs run on device. The schedule keeps the PE streaming back-to-back
512-column bf16 matmuls (~216 ns each, the TRN2 roofline for this GEMM):
  1. Norms: squares+row-sums split between ACT (Square+accum) and DVE
     (tensor_tensor_reduce); ACT sqrt, DVE clamp+reciprocal. inv1 folds the
     16x output scale (sqrt scale=1/256).
  2. inv2 partition-broadcast via fp16 diag matmuls (ones.T @ diag(inv2));
     GPSIMD builds diag tiles, ACT evicts the PSUM result to bf16 bc tiles,
     GPSIMD scales the x2t tiles in place, pipelined ahead of the GEMM.
  3. Main GEMM per (m-tile, column group): one 2-bank PSUM tile [128, 1024]
     accumulates x1t.T @ x2t over 4 K-chunks (bf16, f32 PSUM).
  4. inv1 (row scale) applies during a single-instruction PSUM->SBUF
     eviction per m-tile, alternating DVE tensor_scalar_mul / ACT
     activation(Copy, scale=inv1); bf16 out tiles are DMA'd from the GPSIMD
     queue; host upcasts.
"""

import sys

for _p in ("/root/.axon_site/_ro/trn_rl_repo", "/opt/trn_rl_repo"):
    if _p not in sys.path:
        sys.path.append(_p)

import ml_dtypes
import numpy as np

import concourse.bass as bass
import concourse.tile as tile
from concourse import bacc, mybir
from concourse.bass_utils import run_bass_kernel_spmd
from concourse.masks import make_identity

F32 = mybir.dt.float32
BF16 = mybir.dt.bfloat16
FP16 = mybir.dt.float16
FP8 = mybir.dt.float8e4
P = 128
SCALE = 16.0
EPS = 1e-8

N_CORES = 8
GRID_I = 4  # row-shards of x1
GRID_J = 2  # column-shards of x2
N1 = 8192
N2 = 8192
D = 512
CG = 1024  # output column-group width

_PROGRAM_CACHE = {}


def build_program(n1_local=N1 // GRID_I, n2=N2 // GRID_J, d=D):
    kc = d // P                 # 4 contraction chunks
    m_tiles = n1_local // P     # 16 row tiles per core
    n_cgs = n2 // CG            # 4 column groups
    g1 = n1_local // 512        # 4 x1 norm groups
    g2 = n2 // 512              # 8 x2 norm groups

    nc = bacc.Bacc("TRN2", target_bir_lowering=False, debug=False,
                   num_devices=N_CORES)
    x1t = nc.dram_tensor("x1t", [d, n1_local], BF16, kind="ExternalInput")
    x1n = nc.dram_tensor("x1n", [g1, P, 2048], FP8, kind="ExternalInput")
    x2n = nc.dram_tensor("x2n", [g2, P, 2048], FP8, kind="ExternalInput")
    x2t = nc.dram_tensor("x2t", [d, n2], BF16, kind="ExternalInput")
    out = nc.dram_tensor("out", [n1_local, n2], BF16, kind="ExternalOutput")

    AF = mybir.ActivationFunctionType
    ALU = mybir.AluOpType

    with tile.TileContext(nc) as tc:
        with (
            tc.tile_pool(name="const", bufs=1) as const,
            tc.tile_pool(name="xt", bufs=1) as xt,
            tc.tile_pool(name="ldn", bufs=1) as ldn,
            tc.tile_pool(name="sq", bufs=4) as sqp,
            tc.tile_pool(name="stat", bufs=1) as stat,
            tc.tile_pool(name="dg", bufs=4) as dgp,
            tc.tile_pool(name="bc", bufs=2) as bcp,
            tc.tile_pool(name="ot", bufs=4) as otp,
            tc.tile_pool(name="ps", bufs=7, space="PSUM") as psp,
            tc.tile_pool(name="psb", bufs=1, space="PSUM") as psb,
        ):
            # ---- constants (cheap memsets first so PE warmup starts early) --
            ones_h = const.tile([P, P], FP16)
            nc.gpsimd.memset(ones_h[:], 1.0)
            warm = const.tile([P, 512], FP16)
            nc.gpsimd.memset(warm[:], 0.0)
            ident4 = const.tile([P, 4, P], FP16)
            nc.gpsimd.memset(ident4[:], 0.0)
            for b in range(4):
                make_identity(nc, ident4[:, b], nomemset=True)

            # ---- input DMAs (priority order) --------------------------------
            x1n_t = [ldn.tile([P, 4, 512], FP8, tag=f"x1n_{g}",
                              name=f"x1n_{g}") for g in range(g1)]
            x2n_t = [ldn.tile([P, 4, 512], FP8, tag=f"x2n_{g}",
                              name=f"x2n_{g}") for g in range(g2)]
            x1T = [xt.tile([P, n1_local], BF16, tag=f"x1T_{k}",
                           name=f"x1T_{k}") for k in range(kc)]
            x2T = [[xt.tile([P, CG], BF16, tag=f"x2T_{k}_{cg}",
                            name=f"x2T_{k}_{cg}") for cg in range(n_cgs)]
                   for k in range(kc)]

            for g in (0, 1):
                nc.sync.dma_start(
                    x2n_t[g][:], x2n.ap()[g].rearrange("p (j e) -> p j e", j=4)
                )
            for k in range(kc):
                nc.sync.dma_start(x2T[k][0][:], x2t[k * P:(k + 1) * P, 0:CG])
            for g in range(g1):
                nc.sync.dma_start(
                    x1n_t[g][:], x1n.ap()[g].rearrange("p (j e) -> p j e", j=4)
                )
            for k in range(kc):
                nc.sync.dma_start(x1T[k][:], x1t[k * P:(k + 1) * P, :])
            for cg in range(1, n_cgs):
                for g in (2 * cg, 2 * cg + 1):
                    nc.sync.dma_start(
                        x2n_t[g][:],
                        x2n.ap()[g].rearrange("p (j e) -> p j e", j=4),
                    )
                for k in range(kc):
                    nc.sync.dma_start(
                        x2T[k][cg][:],
                        x2t[k * P:(k + 1) * P, cg * CG:(cg + 1) * CG],
                    )

            # ---- PE warmup against the p-state ramp -------------------------
            for w in range(12):
                ps_w = psb.tile([P, 512], F32, tag="psb", name=f"warm_{w}")
                nc.tensor.matmul(ps_w[:], lhsT=ones_h[:], rhs=warm[:],
                                 start=True, stop=True)

            # ---- stats / broadcast helpers ----------------------------------
            ssq2 = [stat.tile([P, 8], F32, tag=f"ssq2_{cg}", name=f"ssq2_{cg}")
                    for cg in range(n_cgs)]
            inv2 = [stat.tile([P, 8], F32, tag=f"inv2_{cg}", name=f"inv2_{cg}")
                    for cg in range(n_cgs)]
            dg4s = {}
            psbs = {}
            bcs = [bcp.tile([P, CG], BF16, tag="bc", name=f"bc_{cg}")
                   for cg in range(n_cgs)]

            def sq_group(src, acc):
                """acc[:, 0:4] = row sums of src[:, j]^2 (one norm group)."""
                for j in range(4):
                    sq_t = sqp.tile([P, 512], BF16, tag="sq")
                    nc.scalar.activation(
                        sq_t[:], src[:, j], AF.Square,
                        accum_out=acc[:, j:j + 1],
                    )

            def stats_half(cg, h):
                """inv2[cg][:, 4h:4h+4] = 1 / max(row_norm, EPS)."""
                g = 2 * cg + h
                s = ssq2[cg]
                sq_group(x2n_t[g], s[:, 4 * h:4 * h + 4])
                iv = inv2[cg][:, 4 * h:4 * h + 4]
                nc.scalar.activation(iv, s[:, 4 * h:4 * h + 4], AF.Sqrt)
                nc.vector.tensor_scalar_max(iv, iv, EPS)
                nc.vector.reciprocal(iv, iv)

            def dg4_build(cg, h):
                dg4 = dgp.tile([P, 4, P], FP16, tag="dg", name=f"dg_{cg}_{h}")
                nc.vector.tensor_mul(
                    dg4[:], ident4[:],
                    inv2[cg][:, 4 * h:4 * h + 4, None].to_broadcast((P, 4, P)),
                )
                dg4s[(cg, h)] = dg4

            def bcast_mm(cg, h):
                ps_b = psb.tile([P, 512], F32, tag="psb", name=f"psb_{cg}_{h}")
                nc.tensor.matmul(ps_b[:], lhsT=ones_h[:], rhs=dg4s[(cg, h)][:],
                                 start=True, stop=True)
                psbs[(cg, h)] = ps_b

            def bc_copy(cg, h):
                c0 = 4 * h
                nc.vector.tensor_copy(
                    bcs[cg][:, c0 * P:(c0 + 4) * P], psbs[(cg, h)][:]
                )

            def scale_x2(cg, k):
                nc.vector.tensor_mul(x2T[k][cg][:], x2T[k][cg][:], bcs[cg][:])

            ot2s = {}

            def gemm_m(cg, m):
                pss = [psp.tile([P, 512], F32, tag="ps",
                                name=f"ps_{cg}_{m}_{j}") for j in range(2)]
                for k in range(kc):
                    for j in range(2):
                        nc.tensor.matmul(
                            pss[j][:],
                            lhsT=x1T[k][:, m * P:(m + 1) * P],
                            rhs=x2T[k][cg][:, j * 512:(j + 1) * 512],
                            start=(k == 0), stop=(k == kc - 1),
                        )
                if m % 2 == 0:
                    ot2s[cg] = otp.tile([P, 2, CG], BF16, tag="ot",
                                        name=f"ot_{cg}_{m}")
                ot = ot2s[cg][:, m % 2]
                iv = inv1[:, m:m + 1]
                nc.vector.tensor_scalar_mul(ot[0:P, 0:512], pss[0][:], iv)
                nc.scalar.activation(ot[0:P, 512:1024], pss[1][:], AF.Copy,
                                     scale=iv)
                if m % 2 == 1:
                    q = m // 2
                    dst = out.ap()[2 * q * P:(2 * q + 2) * P,
                                   cg * CG:(cg + 1) * CG]
                    nc.scalar.dma_start(
                        dst.rearrange("(t p) c -> p t c", p=P), ot2s[cg][:]
                    )

            # ---- bootstrap: cg0 prep, cg1 stats, x1 stats -------------------
            for h in (0, 1):
                stats_half(0, h)
                dg4_build(0, h)
            for h in (0, 1):
                bcast_mm(0, h)
                bc_copy(0, h)
            for k in range(kc):
                scale_x2(0, k)

            ssq1 = stat.tile([P, 4 * g1], F32, tag="ssq1")
            inv1 = stat.tile([P, 4 * g1], F32, tag="inv1")
            # nrm/16 = sqrt(ssq/256); inv1 = 16/nrm (clamped), in halves so
            # the first m-tiles' evictions unblock early
            for half in range(2):
                for g in range(half * g1 // 2, (half + 1) * g1 // 2):
                    sq_group(x1n_t[g], ssq1[:, 4 * g:4 * g + 4])
                sl = slice(half * 2 * g1, (half + 1) * 2 * g1)
                nc.scalar.activation(inv1[:, sl], ssq1[:, sl], AF.Sqrt,
                                     scale=1.0 / 256.0)
                nc.vector.tensor_scalar_max(inv1[:, sl], inv1[:, sl],
                                            EPS / 16.0)
                nc.vector.reciprocal(inv1[:, sl], inv1[:, sl])

            if n_cgs > 1:
                for h in (0, 1):
                    stats_half(1, h)
                    dg4_build(1, h)

            # ---- main loop: gemm(cg); prep cg+1; stats cg+2 -----------------
            for cg in range(n_cgs):
                nxt = cg + 1
                if cg + 2 < n_cgs:
                    for h in (0, 1):
                        stats_half(cg + 2, h)
                        dg4_build(cg + 2, h)
                for m in range(4):
                    gemm_m(cg, m)
                if nxt < n_cgs:
                    bcast_mm(nxt, 0)
                    bc_copy(nxt, 0)
                for m in range(4, 8):
                    gemm_m(cg, m)
                if nxt < n_cgs:
                    bcast_mm(nxt, 1)
                    bc_copy(nxt, 1)
                    for k in range(kc):
                        scale_x2(nxt, k)
                for m in range(8, m_tiles):
                    gemm_m(cg, m)

    nc.compile()
    return nc


def _get_program():
    key = "default"
    if key not in _PROGRAM_CACHE:
        _PROGRAM_CACHE[key] = build_program()
    return _PROGRAM_CACHE[key]


def _norm_groups(x8: np.ndarray) -> np.ndarray:
    """[G*512, 512] f32 -> [G, 128, 2048] fp8 with rows g*512+j*128+p."""
    g = x8.shape[0] // 512
    r = x8.reshape(g, 4, P, 512).transpose(0, 2, 1, 3).reshape(g, P, 2048)
    return np.ascontiguousarray(r.astype(ml_dtypes.float8_e4m3))


def make_in_maps(x1: np.ndarray, x2: np.ndarray) -> list:
    x1 = np.asarray(x1, dtype=np.float32)
    x2 = np.asarray(x2, dtype=np.float32)
    assert x1.shape == (N1, D) and x2.shape == (N2, D), (x1.shape, x2.shape)
    x1_b = x1.astype(ml_dtypes.bfloat16)
    x2_b = x2.astype(ml_dtypes.bfloat16)
    rows = N1 // GRID_I
    cols = N2 // GRID_J
    x1t_i = [np.ascontiguousarray(x1_b[i * rows:(i + 1) * rows].T)
             for i in range(GRID_I)]
    x1n_i = [_norm_groups(x1[i * rows:(i + 1) * rows]) for i in range(GRID_I)]
    x2t_j = [np.ascontiguousarray(x2_b[j * cols:(j + 1) * cols].T)
             for j in range(GRID_J)]
    x2n_j = [_norm_groups(x2[j * cols:(j + 1) * cols]) for j in range(GRID_J)]
    maps = []
    for c in range(N_CORES):
        i, j = c // GRID_J, c % GRID_J
        maps.append({
            "x1t": x1t_i[i],
            "x1n": x1n_i[i],
            "x2n": x2n_j[j],
            "x2t": x2t_j[j],
        })
    return maps


def kernel(x1: np.ndarray, x2: np.ndarray) -> np.ndarray:
    nc = _get_program()
    in_maps = make_in_maps(x1, x2)
    res = run_bass_kernel_spmd(nc, in_maps, core_ids=list(range(N_CORES)))
    rows = N1 // GRID_I
    cols = N2 // GRID_J
    full = np.empty((N1, N2), dtype=np.float32)
    for c in range(N_CORES):
        i, j = c // GRID_J, c % GRID_J
        full[i * rows:(i + 1) * rows, j * cols:(j + 1) * cols] = \
            res.results[c]["out"]
    return full


if __name__ == "__main__":
    rng = np.random.default_rng(0)
    a = rng.standard_normal((N1, D), dtype=np.float32)
    b = rng.standard_normal((N2, D), dtype=np.float32)
    got = kernel(a, b)
    n1 = np.maximum(np.linalg.norm(a, axis=-1, keepdims=True), EPS)
    n2 = np.maximum(np.linalg.norm(b, axis=-1, keepdims=True), EPS)
    want = SCALE * (a / n1) @ (b / n2).T
    err = np.abs(got - want)
    rel = np.linalg.norm(got - want) / np.linalg.norm(want)
    print(f"max abs err: {err.max():.3e}  rel: {rel:.3e}")

